# revision 5
# baseline (speedup 1.0000x reference)
"""Trainium2 Bass kernel: kNN-graph message passing block (MRConv + sync-BN + ReLU).

Math (per batch sample, matching the reference):
  xf (N, C) node features; dense kNN by squared L2 distance; K=16 (self included).
  gmax = max(max_k xf[idx_k], xf);  y = (we-wo)^T x + wo^T gmax (+b);
  BN training-mode over (B, N) per channel; ReLU.

Distribution: one sample per NeuronCore (8 cores).  BN mean/var partial sums are
all-reduced across cores (768 floats).  b cancels inside BN and is ignored.

v2 pipeline (per core):
  1. PE fp32r: u[i, j] = x_i . x_j - 0.5*||x_j||^2 via two fp32r matmuls per
     448-col tile (ones+ones / hi+lo extra rows carry the -x2/2 term at
     ~1e-5 abs error; fp32r input rounding ~2^-12 -> ~0.1-0.3% neighbor-set
     diffs, ~7e-3 output rel err combined with the f16 forward).
  2. DVE exact top-16: max8/max_index/match_replace8/max8/max_index on f32 u.
  3. Index shuffle to the SWDGE wrapped layout (mask-mul + one PE matmul).
  4. SBUF-source transpose dma_gather (f16, elem 256, 2 queues): neighbor
     features arrive channel-major; no HBM randoms, no PE transposes.
  5. Pool scalar_tensor_tensor running max over the 15 neighbor slots;
     DVE f16 max against x itself finishes gmax.
  6. PE f16 conv matmuls; ACT copies with accum (sum / sum-of-squares);
     AllReduce; scale/shift; fused Relu apply; DMA out f32.
"""

import sys
from collections import defaultdict

import numpy as np

for _p in ("/opt/trn_rl_repo", "/root/.axon_site/_ro/trn_rl_repo"):
    if _p not in sys.path:
        sys.path.insert(0, _p)

import concourse.bass as bass
import concourse.mybir as mybir
import concourse.tile as tile
from concourse import bacc
from concourse.bass_utils import run_bass_kernel_spmd
from concourse.tile import add_dep_helper

B, C, OUT = 8, 192, 384
H = W = 56
N_FULL = H * W  # 3136
K = 16
EPS = 1e-5
NCORES = 8
EP = 256  # padded channels for the f16 gather source (512B per node)

F32 = mybir.dt.float32
F32R = mybir.dt.float32r
F16 = mybir.dt.float16
I16 = mybir.dt.int16
U32 = mybir.dt.uint32
AF = mybir.ActivationFunctionType
ALU = mybir.AluOpType


def build(n=N_FULL, jt=448, chunks=None):
    assert n % jt == 0
    nj = n // jt
    tot = float(B * n)
    nblk = (n + 127) // 128
    npad = nblk * 128
    if chunks is None:
        chunks = [5, 5, 5, 5, 3, 2] if nblk == 25 else [nblk]
    assert sum(chunks) == nblk
    nchunks = len(chunks)
    starts = [sum(chunks[:c]) for c in range(nchunks)]

    # conv j-tile emitted once relc covers its node range (after chunk c's fin)
    conv_after = defaultdict(list)
    for j in range(nj):
        blocks_needed = -(-((j + 1) * jt) // 128)
        for c in range(nchunks):
            if starts[c] + chunks[c] >= blocks_needed:
                conv_after[c].append(j)
                break

    nc = bacc.Bacc("TRN2", target_bir_lowering=False, debug=False,
                   num_swdge_queues=2)
    xc0d = nc.declare_dram_parameter("xc0", [128, n], F32R, isOutput=False)
    c1ad = nc.declare_dram_parameter("c1a", [66, n], F32R, isOutput=False)
    c1bd = nc.declare_dram_parameter("c1b", [66, n], F32R, isOutput=False)
    xsd = nc.declare_dram_parameter("xs", [npad, EP], F16, isOutput=False)
    xhd = nc.declare_dram_parameter("xh", [128, n], F16, isOutput=False)
    xh1d = nc.declare_dram_parameter("xh1", [64, n], F16, isOutput=False)
    wdd = nc.declare_dram_parameter("wd", [C, OUT], F16, isOutput=False)
    wod = nc.declare_dram_parameter("wo", [C, OUT], F16, isOutput=False)
    a8d = nc.declare_dram_parameter("a8", [128, 128], F32, isOutput=False)
    maskd = nc.declare_dram_parameter("maskc", [128, 8], F32, isOutput=False)
    gammad = nc.declare_dram_parameter("gamma", [OUT], F32, isOutput=False)
    betad = nc.declare_dram_parameter("beta", [OUT], F32, isOutput=False)
    yout = nc.declare_dram_parameter("y", [OUT, n], F32, isOutput=True)

    bn_in = nc.dram_tensor("bn_in", [2 * OUT], F32)
    bn_out = nc.dram_tensor("bn_out", [2 * OUT], F32, addr_space="Shared")

    with tile.TileContext(nc) as tc:
        with (
            tc.tile_pool(name="persist", bufs=1) as per,
            tc.tile_pool(name="upool", bufs=2) as upool,
            tc.tile_pool(name="small", bufs=2) as small,
            tc.tile_pool(name="gpool", bufs=4) as gpool,
            tc.tile_pool(name="vpool", bufs=2) as vpool,
            tc.tile_pool(name="fin", bufs=2) as fin,
            tc.tile_pool(name="yst", bufs=2) as yst,
            tc.tile_pool(name="ups", bufs=2, space="PSUM") as ups,
            tc.tile_pool(name="tpsP", bufs=1, space="PSUM") as tpsP,
            tc.tile_pool(name="yps", bufs=2, space="PSUM") as yps,
        ):
            xc0 = per.tile([128, n], F32R, tag="xc0")
            c1a = per.tile([66, n], F32R, tag="c1a")
            c1b = per.tile([66, n], F32R, tag="c1b")
            xs = per.tile([128, nblk, EP], F16, tag="xs")
            xh = per.tile([128, n], F16, tag="xh")
            xh1 = per.tile([64, n], F16, tag="xh1")
            relc0 = per.tile([128, n], F16, tag="relc0")
            relc1 = per.tile([64, n], F16, tag="relc1")
            ypre = [
                per.tile([128, n], F16, tag=f"ypre{i}", name=f"ypre{i}")
                for i in range(3)
            ]
            wd0 = per.tile([128, OUT], F16, tag="wd0")
            wd1 = per.tile([64, OUT], F16, tag="wd1")
            wo0 = per.tile([128, OUT], F16, tag="wo0")
            wo1 = per.tile([64, OUT], F16, tag="wo1")
            a8t = per.tile([128, 128], F32, tag="a8t")
            mskt = per.tile([128, 8], F32, tag="mskt")
            walls = [
                per.tile([128, K * chunks[r] * 8], I16, tag=f"wall{r}",
                         name=f"wall{r}")
                for r in range(nchunks)
            ]
            sums = [per.tile([128, nj], F32, tag=f"s1_{o}", name=f"s1_{o}")
                    for o in range(3)]
            sqs = [per.tile([128, nj], F32, tag=f"s2_{o}", name=f"s2_{o}")
                   for o in range(3)]
            epst = per.tile([128, 1], F32, tag="epst")

            nc.sync.dma_start(out=xc0, in_=xc0d[:, :])
            nc.sync.dma_start(out=c1a, in_=c1ad[:, :])
            nc.sync.dma_start(out=c1b, in_=c1bd[:, :])
            xsf = xsd[:, :]
            nc.sync.dma_start(out=xs, in_=bass.AP(
                tensor=xsf.tensor, offset=xsf.offset,
                ap=[[EP, 128], [128 * EP, nblk], [1, EP]]))
            nc.sync.dma_start(out=xh, in_=xhd[:, :])
            nc.sync.dma_start(out=xh1, in_=xh1d[:, :])
            nc.sync.dma_start(out=wd0, in_=wdd[0:128, :])
            nc.sync.dma_start(out=wd1, in_=wdd[128:192, :])
            nc.sync.dma_start(out=wo0, in_=wod[0:128, :])
            nc.sync.dma_start(out=wo1, in_=wod[128:192, :])
            nc.sync.dma_start(out=a8t, in_=a8d[:, :])
            nc.sync.dma_start(out=mskt, in_=maskd[:, :])
            nc.vector.memset(epst, EPS)
            for wt in walls:
                nc.vector.memset(wt, 0)

            gk_tiles = {}

            def phase1(rb):
                i0 = rb * 128
                m = min(128, n - i0)
                nih = m // 16
                u = upool.tile([128, n], F32, tag="u", name="u")
                for j in range(nj):
                    js = slice(j * jt, (j + 1) * jt)
                    ps = ups.tile([128, jt], F32, tag="ups", name="ups")
                    nc.tensor.matmul(
                        out=ps[:m], lhsT=xc0[:, i0:i0 + m], rhs=xc0[:, js],
                        start=True, stop=False,
                    )
                    nc.tensor.matmul(
                        out=ps[:m], lhsT=c1a[:, i0:i0 + m], rhs=c1b[:, js],
                        start=False, stop=True,
                    )
                    nc.scalar.copy(out=u[:m, js], in_=ps[:m])

                m1 = small.tile([128, 8], F32, tag="m1", name="m1")
                m2 = small.tile([128, 8], F32, tag="m2", name="m2")
                i12 = small.tile([128, 16], U32, tag="i12", name="i12")
                nc.vector.max(out=m1[:m], in_=u[:m])
                nc.vector.max_index(out=i12[:m, 0:8], in_max=m1[:m], in_values=u[:m])
                nc.vector.match_replace(
                    out=u[:m], in_to_replace=m1[:m], in_values=u[:m],
                    imm_value=-1e30,
                )
                nc.vector.max(out=m2[:m], in_=u[:m])
                nc.vector.max_index(out=i12[:m, 8:16], in_max=m2[:m], in_values=u[:m])

                idxf = small.tile([128, 16], F32, tag="idxf", name="idxf")
                nc.vector.tensor_copy(out=idxf[:m], in_=i12[:m])

                bmat = small.tile([128, 128], F32, tag="bmat", name="bmat")
                idx_exp = bass.AP(
                    tensor=idxf.tensor, offset=idxf.offset,
                    ap=[[idxf.ap[0][0], m], [1, K], [0, 8]],
                )
                msk_exp = bass.AP(
                    tensor=mskt.tensor, offset=mskt.offset,
                    ap=[[mskt.ap[0][0], m], [0, K], [1, 8]],
                )
                nc.vector.tensor_mul(out=bmat[:m], in0=idx_exp, in1=msk_exp)
                tps = tpsP.tile([128, 128], F32, tag="tps", name="tps")
                nc.tensor.matmul(
                    out=tps, lhsT=a8t[:m], rhs=bmat[:m], start=True, stop=True
                )
                r = max(c for c in range(nchunks) if starts[c] <= rb)
                rloc = rb - starts[r]
                wt = walls[r]
                dst = bass.AP(
                    tensor=wt.tensor, offset=wt.offset + rloc * 8,
                    ap=[wt.ap[0], [chunks[r] * 8, K], [1, nih]],
                )
                srcap = bass.AP(
                    tensor=tps.tensor, offset=tps.offset,
                    ap=[tps.ap[0], [8, K], [1, nih]],
                )
                nc.vector.tensor_copy(out=dst, in_=srcap)

            def emit_chunk(c):
                """Gathers (queues 0/1) interleaved with Pool running-max."""
                wt = walls[c]
                rc = chunks[c]
                ni = rc * 128
                tiles = []
                vm = vpool.tile([128, 2, ni], F16, tag="vm", name="vm")

                def gather(k):
                    gk = gpool.tile([128, 2, ni], F16, tag="gk", name="gk")
                    nc.gpsimd.dma_gather(
                        gk, xs[:, :, :],
                        wt[:, k * rc * 8:(k + 1) * rc * 8],
                        num_idxs=ni, num_idxs_reg=ni, elem_size=EP,
                        transpose=True, queue_num=k % 2,
                        sbuf_tokens_per_rank=128,
                        sbuf_free_dim_per_rank=2 * EP,
                    )
                    tiles.append(gk)

                gather(1)
                gather(2)
                for j in range(13):
                    gather(j + 3)
                    src = tiles[0] if j == 0 else vm
                    nc.vector.tensor_tensor(
                        out=vm, in0=tiles[j + 1], in1=src, op=ALU.max,
                    )
                nc.vector.tensor_tensor(
                    out=vm, in0=tiles[14], in1=vm, op=ALU.max,
                )
                gk_tiles[c] = vm

            def emit_fin(c):
                vm = gk_tiles[c]
                node0 = starts[c] * 128
                ni = min(chunks[c] * 128, n - node0)
                ns = slice(node0, node0 + ni)
                nc.vector.tensor_tensor(
                    out=relc0[:, ns], in0=vm[:, 0, 0:ni], in1=xh[:, ns],
                    op=ALU.max,
                )
                nc.vector.tensor_tensor(
                    out=relc1[:, ns], in0=vm[0:64, 1, 0:ni], in1=xh1[:, ns],
                    op=ALU.max,
                )

            def emit_conv(j):
                js = slice(j * jt, (j + 1) * jt)
                for oc in range(3):
                    ocs = slice(oc * 128, (oc + 1) * 128)
                    ps = yps.tile([128, jt], F32, tag="yps", name="yps")
                    nc.tensor.matmul(
                        out=ps, lhsT=wd0[:, ocs], rhs=xh[:, js],
                        start=True, stop=False,
                    )
                    nc.tensor.matmul(
                        out=ps, lhsT=wd1[:, ocs], rhs=xh1[:, js],
                        start=False, stop=False,
                    )
                    nc.tensor.matmul(
                        out=ps, lhsT=wo0[:, ocs], rhs=relc0[:, js],
                        start=False, stop=False,
                    )
                    nc.tensor.matmul(
                        out=ps, lhsT=wo1[:, ocs], rhs=relc1[:, js],
                        start=False, stop=True,
                    )
                    nc.scalar.activation(
                        out=ypre[oc][:, js], in_=ps, func=AF.Copy,
                        accum_out=sums[oc][:, j:j + 1],
                    )
                    sq = fin.tile([128, jt], F16, tag="sqscr", name="sqscr")
                    nc.scalar.activation(
                        out=sq, in_=ypre[oc][:, js], func=AF.Square,
                        accum_out=sqs[oc][:, j:j + 1],
                    )

            # ---- emission: topk stream + per-chunk gather/rmax one chunk behind
            for rb in range(nblk):
                phase1(rb)
                for c in range(nchunks):
                    if starts[c] + chunks[c] - 1 == rb:
                        emit_chunk(c)
                        if c >= 1:
                            emit_fin(c - 1)
                            for j in conv_after[c - 1]:
                                emit_conv(j)
            emit_fin(nchunks - 1)
            for j in conv_after[nchunks - 1]:
                emit_conv(j)

            # ---- BN stats + sync + apply ----
            for oc in range(3):
                t1 = fin.tile([128, 1], F32, tag=f"t1_{oc}", name=f"t1_{oc}")
                t2 = fin.tile([128, 1], F32, tag=f"t2_{oc}", name=f"t2_{oc}")
                nc.vector.tensor_reduce(
                    out=t1, in_=sums[oc], axis=mybir.AxisListType.X, op=ALU.add
                )
                nc.vector.tensor_reduce(
                    out=t2, in_=sqs[oc], axis=mybir.AxisListType.X, op=ALU.add
                )
                nc.sync.dma_start(
                    out=bn_in[oc * 128:(oc + 1) * 128].rearrange("(p a) -> p a", a=1),
                    in_=t1,
                )
                nc.sync.dma_start(
                    out=bn_in[OUT + oc * 128:OUT + (oc + 1) * 128].rearrange(
                        "(p a) -> p a", a=1
                    ),
                    in_=t2,
                )

            cc = nc.gpsimd.collective_compute(
                "AllReduce",
                ALU.add,
                ins=[bn_in[:]],
                outs=[bn_out[:]],
                replica_groups=[list(range(NCORES))],
            )

            for oc in range(3):
                ocs = slice(oc * 128, (oc + 1) * 128)
                r1 = fin.tile([128, 1], F32, tag=f"r1_{oc}", name=f"r1_{oc}")
                r2 = fin.tile([128, 1], F32, tag=f"r2_{oc}", name=f"r2_{oc}")
                d1 = nc.sync.dma_start(
                    out=r1,
                    in_=bn_out[oc * 128:(oc + 1) * 128].rearrange("(p a) -> p a", a=1),
                )
                d2 = nc.sync.dma_start(
                    out=r2,
                    in_=bn_out[OUT + oc * 128:OUT + (oc + 1) * 128].rearrange(
                        "(p a) -> p a", a=1
                    ),
                )
                add_dep_helper(d1.ins, cc.ins, reason="allreduce before readback")
                add_dep_helper(d2.ins, cc.ins, reason="allreduce before readback")

                g_t = fin.tile([128, 1], F32, tag=f"g_{oc}", name=f"g_{oc}")
                b_t = fin.tile([128, 1], F32, tag=f"b_{oc}", name=f"b_{oc}")
                nc.sync.dma_start(
                    out=g_t, in_=gammad[ocs].rearrange("(p a) -> p a", a=1)
                )
                nc.sync.dma_start(
                    out=b_t, in_=betad[ocs].rearrange("(p a) -> p a", a=1)
                )

                mean = fin.tile([128, 1], F32, tag=f"mean_{oc}", name=f"mean_{oc}")
                msq = fin.tile([128, 1], F32, tag=f"msq_{oc}", name=f"msq_{oc}")
                var = fin.tile([128, 1], F32, tag=f"var_{oc}", name=f"var_{oc}")
                rstd = fin.tile([128, 1], F32, tag=f"rstd_{oc}", name=f"rstd_{oc}")
                scl = fin.tile([128, 1], F32, tag=f"scl_{oc}", name=f"scl_{oc}")
                shf = fin.tile([128, 1], F32, tag=f"shf_{oc}", name=f"shf_{oc}")
                nc.scalar.mul(out=mean, in_=r1, mul=1.0 / tot)
                nc.scalar.mul(out=msq, in_=r2, mul=1.0 / tot)
                nc.vector.tensor_mul(out=var, in0=mean, in1=mean)
                nc.vector.tensor_sub(out=var, in0=msq, in1=var)
                nc.scalar.activation(
                    out=rstd, in_=var, func=AF.Sqrt, bias=epst, scale=1.0
                )
                nc.vector.reciprocal(out=rstd, in_=rstd)
                nc.vector.tensor_mul(out=scl, in0=g_t, in1=rstd)
                nc.vector.tensor_mul(out=shf, in0=mean, in1=scl)
                nc.vector.tensor_sub(out=shf, in0=b_t, in1=shf)

                ys_t = yst.tile([128, n], F32, tag="ystage", name="ystage")
                nc.scalar.activation(
                    out=ys_t, in_=ypre[oc], func=AF.Relu, bias=shf, scale=scl
                )
                nc.sync.dma_start(out=yout[ocs, :], in_=ys_t)

    nc.compile()
    return nc


_NC_CACHE = {}


def _get_nc():
    if "nc" not in _NC_CACHE:
        _NC_CACHE["nc"] = build()
    return _NC_CACHE["nc"]


def _round_mant(a, bits):
    a = np.asarray(a, np.float32).copy()
    v = a.view(np.uint32)
    shift = 23 - bits
    v += (1 << (shift - 1)) + ((v >> shift) & 1)
    v &= np.uint32(~((1 << shift) - 1) & 0xFFFFFFFF)
    return a


def make_in_maps(x, w, gamma, beta, n=N_FULL):
    x = np.ascontiguousarray(np.asarray(x, np.float32))
    w = np.asarray(w, np.float32)
    nb = x.shape[0]
    npad = ((n + 127) // 128) * 128
    xTf = x.reshape(nb, C, n)
    x2 = np.sum(xTf * xTf, axis=1)          # (B, n)
    mx2 = -0.5 * x2
    hi = _round_mant(mx2, 10)
    lo = (mx2 - hi).astype(np.float32)
    ones = np.ones((1, n), np.float32)

    xs = np.zeros((nb, npad, EP), np.float16)
    xs[:, :n, :C] = xTf.transpose(0, 2, 1)

    we = w[:, 0::2]
    woh = w[:, 1::2]
    wd_h = np.ascontiguousarray((we - woh).T.astype(np.float16))
    wo_h = np.ascontiguousarray(woh.T.astype(np.float16))
    jj = np.arange(128)
    a8_h = (jj[:, None] % 16 == jj[None, :] % 16).astype(np.float32)
    mask_h = (jj[:, None] // 16 == np.arange(8)[None, :]).astype(np.float32)
    g = np.ascontiguousarray(np.asarray(gamma, np.float32))
    bt = np.ascontiguousarray(np.asarray(beta, np.float32))

    maps = []
    for k in range(nb):
        xT = xTf[k]
        c1a_h = np.concatenate([xT[128:192], ones, ones], 0).astype(np.float32)
        c1b_h = np.concatenate(
            [xT[128:192], hi[k:k + 1], lo[k:k + 1]], 0
        ).astype(np.float32)
        maps.append({
            "xc0": np.ascontiguousarray(xT[0:128]),
            "c1a": np.ascontiguousarray(c1a_h),
            "c1b": np.ascontiguousarray(c1b_h),
            "xs": np.ascontiguousarray(xs[k]),
            "xh": np.ascontiguousarray(xT[0:128].astype(np.float16)),
            "xh1": np.ascontiguousarray(xT[128:192].astype(np.float16)),
            "wd": wd_h,
            "wo": wo_h,
            "a8": a8_h,
            "maskc": mask_h,
            "gamma": g,
            "beta": bt,
        })
    return maps


def _run_with_retry(nc, in_maps, tries=4):
    """The axon-tunneled device occasionally reports
    NRT_EXEC_UNIT_UNRECOVERABLE and recovers after a few minutes; retry with
    a backend reset instead of failing the whole call."""
    import time

    for attempt in range(tries):
        try:
            return run_bass_kernel_spmd(nc, in_maps, list(range(NCORES))).results
        except Exception:
            if attempt == tries - 1:
                raise
            try:
                import jax

                jax.clear_backends()
            except Exception:
                pass
            time.sleep(90)


def kernel(x, w, b, gamma, beta):
    del b  # bias cancels inside training-mode BatchNorm
    nc = _get_nc()
    in_maps = make_in_maps(x, w, gamma, beta)
    res = _run_with_retry(nc, in_maps)
    y = np.stack([np.asarray(res[k]["y"]) for k in range(B)], axis=0)
    return y.reshape(B, OUT, H, W).astype(np.float32)
